# revision 25
# baseline (speedup 1.0000x reference)
"""Multi-head attention TRN2 Bass kernel (8 NeuronCores).

Problem: B=4, S=2048, D_MODEL=1024, H=16, d_k=d_v=64 (fp32 in/out).

Sharding: core c handles batch b=c//2 and head-half hh=c%2 (8 heads).
Each core computes partial_out = softmax(qh@khT/8) @ vh @ Wo[rows of its
heads]; the host sums the two partials per batch.

Host prep: q/k/v are cast to fp16 and transposed to [D, S] per batch,
weights cast to fp16, so the device only does matmul-layout loads.

On-core dataflow (fp16 matmuls, fp32 PSUM accumulation):
  - qhT/khT computed in [d, S] layout (2 heads per 128-partition tile)
  - scores computed transposed [Sk, Sq]; the two K=64 head matmuls of a
    pair run concurrently in PE row groups (base partition 0/64)
  - exp on ACT with the 1/sqrt(dk) scale fused; no max subtraction
  - AV stationary is [ones64 | vh]: PSUM rows 0:64 get the softmax
    denominator broadcast 64-wide, rows 64:128 the unnormalized out
  - normalize via one approx-reciprocal + two muls into fp16 stk tiles
  - Wo accumulates the 4 head-pair stk tiles (K=128 each) into fp32

Scheduling (v4):
  - ~18 throwaway matmuls on scratch SBUF warm the PE's HAM clock gate
    during the DMA head, so the first real matmuls run at 2.4 GHz
  - DMA priority kT.h0 -> qT.h0 -> kT.h1 -> vT -> (qT.h1 at sq=1);
    2 KB lines; khT pair0 is projected half-by-half so the first
    attention block starts ~18us in
  - every other projection chunk (vh, khT pairs 1-3, qhT) is filler
    emitted BETWEEN the attention matmuls, ordered by DMA arrival
  - attention emits in 2-skt groups [sc,sc | fillers | av x4 lagged] so
    same-shape matmul runs hide LDWEIGHTS; each block's AV tail +
    normalize is deferred into the NEXT block's first group (prologue),
    removing the ACT bubble at every block boundary
  - Wo chunks are spread one sq-block late as fillers
"""

import numpy as np

import concourse.bass as bass  # noqa: F401
import concourse.mybir as mybir
import concourse.tile as tile
from concourse import bacc
from concourse.bass_utils import run_bass_kernel_spmd

S = 2048  # sequence length
D = 1024  # d_model
HPC = 8  # heads per core
DK = 64  # head dim
HD = HPC * DK  # 512: projected width per core
N_CORES = 8

SB = S // 512  # 4 s-blocks of 512
KT = D // 128  # 8 contraction tiles for projections
SKT = S // 128  # 16 key tiles
NP = HPC // 2  # 4 head pairs
F32 = mybir.dt.float32
F16 = mybir.dt.float16

_CACHE = {}


def _build():
    nc = bacc.Bacc("TRN2", target_bir_lowering=False, debug=False, num_devices=N_CORES)
    qT = nc.dram_tensor("qT", [D, S], F16, kind="ExternalInput")
    kT = nc.dram_tensor("kT", [D, S], F16, kind="ExternalInput")
    vT = nc.dram_tensor("vT", [D, S], F16, kind="ExternalInput")
    wq = nc.dram_tensor("wq", [D, HD], F16, kind="ExternalInput")
    wk = nc.dram_tensor("wk", [D, HD], F16, kind="ExternalInput")
    wv = nc.dram_tensor("wv", [D, HD], F16, kind="ExternalInput")
    wo = nc.dram_tensor("wo", [HD, D], F16, kind="ExternalInput")
    out = nc.dram_tensor("out", [S, D], F16, kind="ExternalOutput")

    with tile.TileContext(nc) as tc:
        with (
            tc.tile_pool(name="resident", bufs=1) as resident,
            tc.tile_pool(name="kstage", bufs=2) as kstagep,
            tc.tile_pool(name="ostage", bufs=3) as ostagep,
            tc.tile_pool(name="et", bufs=8) as etp,
            tc.tile_pool(name="misc", bufs=1) as misc,
            tc.tile_pool(name="stk", bufs=6) as stkp,
            tc.tile_pool(name="outst", bufs=2) as outstp,
            tc.tile_pool(name="ps_sc", bufs=2, space="PSUM") as ps_sc,
            tc.tile_pool(name="ps_av", bufs=1, space="PSUM") as ps_av,
            tc.tile_pool(name="ps_pj", bufs=2, space="PSUM") as ps_pj,
        ):
            # --- resident tiles ---
            wv16 = resident.tile([128, KT, HD], F16)
            wk16 = resident.tile([128, KT, HD], F16)
            wq16 = resident.tile([128, KT, HD], F16)
            wo16 = resident.tile([128, HD // 128, D], F16)
            qhT = resident.tile([128, NP, S], F16)  # [2-head tile, pair, Sq]
            khT = resident.tile([128, NP, S], F16)
            scratch = resident.tile([128, 512], F16)
            nc.vector.memset(scratch[:, :], 0.0)
            # AV stationary: [..., 0:64] = 1.0 (denominator), [..., 64:128] = vh
            vh = resident.tile([128, SKT, HPC, 128], F16)
            nc.vector.memset(vh[:, :, :, 0:DK], 1.0)

            # --- PE warm-up: junk matmuls while the first DMAs land ---
            wups = ps_pj.tile([128, 512], F32, tag="pj")
            for _ in range(18):
                nc.tensor.matmul(
                    wups[:, :], lhsT=scratch[:, 0:128], rhs=scratch[:, :],
                    start=True, stop=True,
                )

            def stage_half(srcT, pool, h):
                """Stage cols [h*1024, (h+1)*1024) of srcT: 2 KB DMA lines."""
                src = srcT.ap().rearrange("(t p) s -> p t s", p=128)
                st = pool.tile([128, KT, 1024], F16, tag="st")
                for t in range(KT):
                    nc.sync.dma_start(
                        out=st[:, t, :], in_=src[:, t, h * 1024 : (h + 1) * 1024]
                    )
                return st

            def sb_slice(units, sb):
                return units[sb // 2], slice((sb % 2) * 512, (sb % 2) * 512 + 512)

            # ---- filler chunks: ~8 matmuls + 1 copy each ----
            def pj_chunk(units, w16, dstT, m, sb):
                def emit():
                    st, cols = sb_slice(units, sb)
                    ps = ps_pj.tile([128, 512], F32, tag="pj")
                    for t in range(KT):
                        nc.tensor.matmul(
                            ps[:, :],
                            lhsT=w16[:, t, m * 128 : (m + 1) * 128],
                            rhs=st[:, t, cols],
                            start=(t == 0),
                            stop=(t == KT - 1),
                        )
                    nc.vector.tensor_copy(
                        dstT[:, m, sb * 512 : (sb + 1) * 512], ps[:, :]
                    )

                return emit

            def pv_chunk(vunits, skt):
                """vh[:, skt, :, 64:128] from v-cols [skt*128, +128)."""

                def emit():
                    ps = ps_pj.tile([128, 512], F32, tag="pj")
                    vcols = slice((skt % 8) * 128, (skt % 8) * 128 + 128)
                    for t in range(KT):
                        nc.tensor.matmul(
                            ps[:, :],
                            lhsT=vunits[skt // 8][:, t, vcols],
                            rhs=wv16[:, t, :],
                            start=(t == 0),
                            stop=(t == KT - 1),
                        )
                    nc.vector.tensor_copy(
                        vh[:, skt, :, DK:128],
                        ps[:, :].rearrange("p (h d) -> p h d", h=HPC),
                    )

                return emit

            outstates = {}
            stks_by_sq = {}

            def wo_half(sq, chunk, nh):
                def emit():
                    stks = stks_by_sq[sq]
                    mrange = slice(chunk * 128, (chunk + 1) * 128)
                    wops = ps_pj.tile([128, 512], F32, tag="pj")
                    for pair in range(NP):
                        nc.tensor.matmul(
                            wops[:, :],
                            lhsT=stks[pair][:, mrange],
                            rhs=wo16[:, pair, nh * 512 : (nh + 1) * 512],
                            start=(pair == 0),
                            stop=(pair == NP - 1),
                        )
                    outst = outstates[(sq, chunk)]
                    nc.vector.tensor_copy(outst[:, nh, :], wops[:, :])
                    row0 = sq * 512 + chunk * 128
                    for half in range(2):
                        rows = slice(row0 + half * 64, row0 + half * 64 + 64)
                        nc.sync.dma_start(
                            out=out.ap()[rows, nh * 512 : (nh + 1) * 512],
                            in_=outst[half * 64 : half * 64 + 64, nh, :],
                        )

                return emit

            def wo_fillers(sq):
                fs = []
                for chunk in range(4):
                    outst = outstp.tile([128, 2, 512], F16, tag="outst")
                    outstates[(sq, chunk)] = outst
                    for nh in range(2):
                        fs.append(wo_half(sq, chunk, nh))
                return fs

            fillers = []
            pending_tail = [None]

            def attention_block(sq, pair, rate, av_lag_groups=1, prologue=None):
                """One (sq, pair) block in 2-skt groups. The block's own AV
                tail + normalize is DEFERRED: stored in pending_tail and
                emitted by the next block's first group (or explicitly)."""
                cols = slice(sq * 512, (sq + 1) * 512)
                avt = []  # lazily allocated accumulator
                pend = []  # (et, skt) awaiting AV emission
                budget = 0.0

                def av_mms(et, skt):
                    if not avt:
                        av_ = ps_av.tile([128, 1024], F32, tag="av")
                        avt.append(av_)
                    av = avt[0]
                    for x in range(2):
                        nc.tensor.matmul(
                            av[:, x * 512 : (x + 1) * 512],
                            lhsT=vh[:, skt, 2 * pair + x, :],
                            rhs=et[:, x, :],
                            start=(skt == 0),
                            stop=(skt == SKT - 1),
                        )

                for g in range(SKT // 2):
                    for skt in (2 * g, 2 * g + 1):
                        scps = ps_sc.tile([128, 1024], F32, tag="sc")
                        kcols = slice(skt * 128, (skt + 1) * 128)
                        # scores feed the ACT critical path: in the static
                        # schedule a ready score-pair must preempt queued
                        # filler chunks, so pin it to the highest priority
                        with tc.high_priority():
                            nc.tensor.matmul(
                                scps[:, 0:512],
                                lhsT=khT[0:64, pair, kcols],
                                rhs=qhT[0:64, pair, cols],
                                start=True,
                                stop=True,
                            )
                            nc.tensor.matmul(
                                scps[:, 512:1024],
                                lhsT=khT[64:128, pair, kcols],
                                rhs=qhT[64:128, pair, cols],
                                start=True,
                                stop=True,
                            )
                        et = etp.tile([128, 2, 512], F16)
                        nc.scalar.activation(
                            et.rearrange("p a b -> p (a b)"),
                            scps[:, :],
                            mybir.ActivationFunctionType.Exp,
                            scale=1.0 / np.sqrt(DK).item(),
                        )
                        pend.append((et, skt))
                    if g == 0:
                        if pending_tail[0] is not None:
                            pending_tail[0]()
                        if prologue is not None:
                            prologue()
                    budget += 2 * rate
                    while fillers and budget >= 1.0:
                        fillers.pop(0)()
                        budget -= 1.0
                    # gradually taper the AV lag near the block end so the
                    # deferred tail stays small
                    lag = max(1, min(av_lag_groups, SKT // 2 - 2 - g))
                    while len(pend) > 2 * lag:
                        av_mms(*pend.pop(0))

                def tail():
                    for p in pend:
                        av_mms(*p)
                    av = avt[0]
                    rcp = misc.tile([128, 1024], F32, tag="rcp")
                    nc.vector.reciprocal_approx_fast(
                        out=rcp[0:64, :], in_=av[0:64, :]
                    )
                    stk = stkp.tile([128, 512], F16, tag="stk")
                    nc.vector.tensor_mul(
                        stk[0:64, :], av[64:128, 0:512], rcp[0:64, 0:512]
                    )
                    nc.vector.tensor_mul(
                        stk[64:128, :], av[64:128, 512:1024], rcp[0:64, 512:1024]
                    )
                    stks_by_sq.setdefault(sq, {})[pair] = stk
                    pending_tail[0] = None

                pending_tail[0] = tail

            # --- emission ---
            # scalar HWDGE queue: the four weight tensors.
            nc.scalar.dma_start(
                out=wk16, in_=wk.ap().rearrange("(t p) m -> p t m", p=128)
            )
            nc.scalar.dma_start(
                out=wq16, in_=wq.ap().rearrange("(t p) m -> p t m", p=128)
            )
            nc.scalar.dma_start(
                out=wv16, in_=wv.ap().rearrange("(t p) m -> p t m", p=128)
            )
            nc.scalar.dma_start(
                out=wo16, in_=wo.ap().rearrange("(t p) n -> p t n", p=128)
            )
            # sync queue, in critical-path order.
            kunits = {0: stage_half(kT, kstagep, 0)}
            qunits = {0: stage_half(qT, ostagep, 0)}
            kunits[1] = stage_half(kT, kstagep, 1)
            vunits = {0: stage_half(vT, ostagep, 0), 1: stage_half(vT, ostagep, 1)}

            # urgent projections: khT (pair0, kT half0), qhT (pair0, sb0)
            pj_chunk(kunits, wk16, khT, 0, 0)()
            pj_chunk(kunits, wk16, khT, 0, 1)()
            pj_chunk(qunits, wq16, qhT, 0, 0)()

            # block(0,0) fillers in DMA-arrival order: khT (pair0, half1)
            # first (sc(8..15) needs it), then khT pair1, vh as vT lands,
            # qhT(pair1, sb0).
            fillers += [pj_chunk(kunits, wk16, khT, 0, sb) for sb in (2, 3)]
            fillers += [pj_chunk(kunits, wk16, khT, 1, sb) for sb in range(SB)]
            fillers += [pv_chunk(vunits, skt) for skt in range(SKT)]
            fillers.append(pj_chunk(qunits, wq16, qhT, 1, 0))

            attention_block(0, 0, rate=23 / 14, av_lag_groups=3)
            for m in range(1, NP):
                if m + 1 < NP:
                    fillers += [
                        pj_chunk(kunits, wk16, khT, m + 1, sb) for sb in range(SB)
                    ]
                    fillers.append(pj_chunk(qunits, wq16, qhT, m + 1, 0))
                else:
                    fillers += [pj_chunk(qunits, wq16, qhT, p, 1) for p in range(NP)]
                attention_block(0, m, rate=max(len(fillers) / SKT, 0.001))

            # steady state: per sq, the 4 blocks host the previous sq's 8 Wo
            # half-chunks (appended in the first block's prologue, after the
            # last stk of sq-1 materializes) + the next sq's 4 qhT chunks.
            for sq in range(1, SB):
                if sq + 1 < SB and (sq + 1) // 2 not in qunits:
                    qunits[(sq + 1) // 2] = stage_half(qT, ostagep, (sq + 1) // 2)

                def prologue(sq=sq):
                    fillers.extend(wo_fillers(sq - 1))
                    if sq + 1 < SB:
                        fillers.extend(
                            pj_chunk(qunits, wq16, qhT, p, sq + 1) for p in range(NP)
                        )

                attention_block(sq, 0, rate=12 / (4 * SKT), prologue=prologue)
                for pair in range(1, NP):
                    attention_block(
                        sq,
                        pair,
                        rate=max(len(fillers) / (SKT * (NP - pair)), 0.001),
                    )
                # the just-appended wo/qhT fillers drain across this sq
            # final tail: last block's AVs + normalize, then Wo(sq=3)
            pending_tail[0]()
            for f in wo_fillers(SB - 1):
                f()

    nc.compile()
    return nc


def _get_nc():
    if "nc" not in _CACHE:
        _CACHE["nc"] = _build()
    return _CACHE["nc"]


def build_in_maps(q, k, v, Wq, Wk, Wv, Wo):
    """Host prep: shard, cast fp16, pre-transpose activations to [D, S]."""
    q = np.asarray(q, dtype=np.float32)
    k = np.asarray(k, dtype=np.float32)
    v = np.asarray(v, dtype=np.float32)
    wq16 = np.asarray(Wq, dtype=np.float32).astype(np.float16)
    wk16 = np.asarray(Wk, dtype=np.float32).astype(np.float16)
    wv16 = np.asarray(Wv, dtype=np.float32).astype(np.float16)
    wo16 = np.asarray(Wo, dtype=np.float32).astype(np.float16)
    qT = [np.ascontiguousarray(q[b].T).astype(np.float16) for b in range(4)]
    kTt = [np.ascontiguousarray(k[b].T).astype(np.float16) for b in range(4)]
    vTt = [np.ascontiguousarray(v[b].T).astype(np.float16) for b in range(4)]
    in_maps = []
    for c in range(N_CORES):
        b, hh = c // 2, c % 2
        sl = slice(hh * HD, (hh + 1) * HD)
        in_maps.append(
            {
                "qT": qT[b],
                "kT": kTt[b],
                "vT": vTt[b],
                "wq": np.ascontiguousarray(wq16[:, sl]),
                "wk": np.ascontiguousarray(wk16[:, sl]),
                "wv": np.ascontiguousarray(wv16[:, sl]),
                "wo": np.ascontiguousarray(wo16[sl, :]),
            }
        )
    return in_maps


def kernel(q, k, v, Wq, Wk, Wv, Wo):
    nc = _get_nc()
    in_maps = build_in_maps(q, k, v, Wq, Wk, Wv, Wo)
    res = run_bass_kernel_spmd(nc, in_maps, core_ids=list(range(N_CORES)))
    outs = [res.results[c]["out"].astype(np.float32) for c in range(N_CORES)]
    return np.stack([outs[2 * b] + outs[2 * b + 1] for b in range(4)], axis=0)
